# revision 22
# baseline (speedup 1.0000x reference)
"""Trainium2 Bass kernel for nn_BaltNet (2-layer ConvLSTM + decoder + MLP head).

Sharding: data-parallel over batch B=8 (one sample per NeuronCore) for the
recurrent conv part; FC1's [131072, 256] contraction is K-sharded 8 ways
(AllToAll of the decoder features, per-core partial matmul, ReduceScatter).

Layout notes
------------
Conv is computed as matmuls over a zero-padded spatial layout [C, 66, 68]
(1-row halo top/bottom, cols 2..65 interior) so every 3x3 tap is a pure
free-dim offset.  The three vertical taps (ky) are packed into the matmul
contraction dim by keeping row-shifted copies of the input stacked on
partitions; the three horizontal taps (kx) are separate accumulating matmul
passes with shifted column windows.

  A   [105, 66, 68]: layer-0 rhs, 3 groups of (h0[32] + x[3]) at ky=0,-1,+1
      (base group first: engine writes need 32-aligned partition starts)
  Ba  [128, 66, 68]: layer-1 rhs, groups (h0+h1)[64] at ky=-1 (p0-63), ky=0
  Bb  [ 64, 66, 68]: layer-1 rhs, group  (h0+h1)[64] at ky=+1 (kx=0 pass)
  Bb2 [128, 66, 68]: layer-1 rhs, (h0+h1)[64] at (ky=+1,kx=-1) on p0-63 and
      (ky=+1,kx=+1) on p64-127 -- kx pre-baked so layer 1 runs in 5 matmul
      passes per 512-col group instead of 6.

Gates: z = [i f o g] on 128 partitions; g-gate weights/bias pre-scaled x2 so
tanh(g) = 2*sigmoid(2g) - 1 and one Sigmoid covers all 128 partitions.
Pointwise + activations run at half-image granularity (free size 2048) to
amortize per-instruction overhead and halve instruction counts.
Everything 16-bit is fp16 (verified ~1.3e-3 end-to-end vs fp32 reference).
"""

import os
import sys

for _p in ("/opt/trn_rl_repo",):
    if _p not in sys.path and os.path.isdir(_p):
        sys.path.insert(0, _p)

import numpy as np

import concourse.bass as bass
import concourse.mybir as mybir
import concourse.tile as tile
from concourse import bacc
from concourse.bass_utils import run_bass_kernel_spmd

F16 = mybir.dt.float16
F32 = mybir.dt.float32
AF = mybir.ActivationFunctionType
OP = mybir.AluOpType

B, T, C, HID, H, W = 8, 24, 3, 32, 64, 64
G4 = 4 * HID            # 128 gate channels
PH, PW = H + 2, W + 4   # padded spatial: rows 0..65, interior cols 2..65
NPIX = H * W            # 4096
HPIX = NPIX // 2        # 2048 (one 32-row half)
KSL = HID * NPIX // 8   # 16384 per-core FC1 K-slice
N_CORES = 8

TRACE = False           # test.py flips this for profiled runs
_CACHE = {}


def _build_nc():
    nc = bacc.Bacc("TRN2", target_bir_lowering=False, debug=False,
                   num_devices=N_CORES)

    # ---- I/O -------------------------------------------------------------
    xp_d = nc.dram_tensor("xp", [T, C, PH, PW], F16, kind="ExternalInput")
    w0_d = nc.dram_tensor("w0", [105, 3 * G4], F16, kind="ExternalInput")
    w1a_d = nc.dram_tensor("w1a", [128, 3 * G4], F16, kind="ExternalInput")
    w1b2_d = nc.dram_tensor("w1b2", [128, G4], F16, kind="ExternalInput")
    w1bp_d = nc.dram_tensor("w1bp", [64, G4], F16, kind="ExternalInput")
    wd_d = nc.dram_tensor("wd", [105, 3 * G4], F16, kind="ExternalInput")
    b0_d = nc.dram_tensor("b0", [G4, 1], F32, kind="ExternalInput")
    b1_d = nc.dram_tensor("b1", [G4, 1], F32, kind="ExternalInput")
    bd_d = nc.dram_tensor("bd", [G4, 1], F32, kind="ExternalInput")
    fw_d = nc.dram_tensor("fw", [128, 128 * 256], F16, kind="ExternalInput")
    fb_d = nc.dram_tensor("fb", [128, 2], F32, kind="ExternalInput")
    w2_d = nc.dram_tensor("w2", [128, 2 * 97], F16, kind="ExternalInput")
    b2_d = nc.dram_tensor("b2", [97, 1], F32, kind="ExternalInput")
    out_d = nc.dram_tensor("out", [97, 1], F32, kind="ExternalOutput")

    with tile.TileContext(nc) as tc:
        with (
            tc.tile_pool(name="state", bufs=1) as state,
            tc.tile_pool(name="const", bufs=1) as const,
            tc.tile_pool(name="sgate", bufs=3) as sgate,
            tc.tile_pool(name="scr", bufs=3) as scr,
            tc.tile_pool(name="psum", bufs=2, space="PSUM") as psum,
            tc.tile_pool(name="fwp", bufs=2) as fwp,
            tc.tile_pool(name="dram", bufs=1, space="DRAM") as dram,
        ):
            # ---- persistent SBUF state ----------------------------------
            # L1's rhs buffers are double-buffered by step parity: h0(t)'s
            # shifted copies land in set t%2 while L1(t-1)'s matmuls read
            # set (t-1)%2, so the copies never WAR-wait on L1's reads (that
            # wait was the step-period serializer).
            A = state.tile([105, PH, PW], F16)    # L0 rhs (h0 + x), 3 ky-groups
            Ba = [state.tile([128, PH, PW], F16, name=f"Ba{p}")
                  for p in range(2)]              # L1 rhs ky=-1,0
            Bb = [state.tile([64, PH, PW], F16, name=f"Bb{p}")
                  for p in range(2)]              # L1 rhs ky=+1, kx=0
            Bb2 = [state.tile([128, PH, PW], F16, name=f"Bb2{p}")
                   for p in range(2)]             # L1 rhs ky=+1, kx=-1/+1
            # c-state lives on partitions 32-63 so TT ops pair with S[32:64]
            cst0 = state.tile([64, NPIX], F16)
            cst1 = state.tile([64, NPIX], F16)
            hdc = state.tile([HID, NPIX], F16)    # decoder h (feat)

            # ---- constants ----------------------------------------------
            w0 = const.tile([105, 3 * G4], F16)
            w1a = const.tile([128, 3 * G4], F16)
            w1b2 = const.tile([128, G4], F16)
            w1bp = const.tile([64, G4], F16)
            wd = const.tile([105, 3 * G4], F16)
            b0 = const.tile([G4, 1], F32)
            b1 = const.tile([G4, 1], F32)
            bd = const.tile([G4, 1], F32)
            fb = const.tile([128, 2], F32)
            w2 = const.tile([128, 2 * 97], F16)
            b2 = const.tile([97, 1], F32)
            ft = const.tile([128, 8, 128], F16)   # A2A result, FC1 lhsT tiles

            for dst, src in ((w0, w0_d), (w1a, w1a_d), (w1b2, w1b2_d),
                             (w1bp, w1bp_d), (wd, wd_d), (b0, b0_d),
                             (b1, b1_d), (bd, bd_d), (fb, fb_d), (w2, w2_d),
                             (b2, b2_d)):
                nc.sync.dma_start(out=dst[:], in_=src[:])

            # zero-init state (h=0, c=0, halos=0)
            nc.gpsimd.memset(A[:], 0.0)
            for p in range(2):
                nc.gpsimd.memset(Ba[p][:], 0.0)
                nc.gpsimd.memset(Bb[p][:], 0.0)
                nc.gpsimd.memset(Bb2[p][:], 0.0)
            nc.vector.memset(cst0[:], 0.0)
            nc.vector.memset(cst1[:], 0.0)

            # ---- DRAM bounce buffers for collectives --------------------
            a2a_in = dram.tile([HID, NPIX], F16)
            a2a_out = dram.tile([8, 128, 128], F16)
            z1part = dram.tile([8, 256], F32)
            z1red = dram.tile([256], F32)

            def conv_mm_sigma(passes, bias, S):
                """passes: list of (buf, K, lhsT, kx).  Per 32-row half:
                accumulate all passes into a [128,2048] PSUM tile (4 banks,
                four 512-col matmul targets), then sigmoid into S.  The
                pointwise is emitted separately (software pipelining): each
                engine queue is strict FIFO, so the emission order controls
                head-of-line blocking."""
                npass = len(passes)
                for rh in range(2):
                    pz = psum.tile([G4, HPIX], F32, tag="z", name=f"pz{rh}")
                    for ip, (buf, K, lhs, kx) in enumerate(passes):
                        for o in range(4):
                            r0 = 32 * rh + 8 * o
                            rhs = buf[0:K, r0 + 1:r0 + 9, 2 + kx:66 + kx]
                            nc.tensor.matmul(
                                pz[:, 512 * o:512 * o + 512],
                                lhs, rhs, start=(ip == 0),
                                stop=(ip == npass - 1))
                    nc.scalar.activation(
                        out=S[:, rh * HPIX:(rh + 1) * HPIX],
                        in_=pz[:], func=AF.Sigmoid,
                        bias=bias[:, 0:1], scale=1.0)

            def pointwise_core(S, cst):
                """Gate combine for both halves: c' = sig(f)*c + sig(i)*tg
                and tanh(c').  TT inputs must share a base partition, so
                scratch tensors are placed at the base of the gate they
                pair with.  Returns the tanh tiles for the h-muls."""
                ths = []
                for rh in range(2):
                    sl = slice(rh * HPIX, (rh + 1) * HPIX)
                    # tg = 2*sigmoid(2g) - 1, re-based to partitions 0-31
                    tgt = scr.tile([32, HPIX], F16, tag="tgt")
                    nc.vector.tensor_scalar(
                        out=tgt[:], in0=S[96:128, sl],
                        scalar1=2.0, scalar2=-1.0, op0=OP.mult, op1=OP.add)
                    uv = scr.tile([32, 2, HPIX], F16, tag="uv")
                    nc.vector.tensor_mul(uv[:, 0, :], S[0:32, sl], tgt[:])
                    nc.vector.tensor_mul(uv[:, 1, :], S[32:64, sl],
                                         cst[32:64, sl])
                    nc.vector.tensor_add(cst[32:64, sl], uv[:, 0, :],
                                         uv[:, 1, :])
                    tht = scr.tile([96, HPIX], F16, tag="tht")
                    nc.scalar.activation(out=tht[64:96, :],
                                         in_=cst[32:64, sl], func=AF.Tanh)
                    ths.append(tht)
                return ths

            def h_muls(S, ths, hdst):
                for rh in range(2):
                    sl = slice(rh * HPIX, (rh + 1) * HPIX)
                    if hdst is hdc:
                        dst = hdc[:, sl]
                    else:
                        buf, p0 = hdst
                        dst = buf[p0:p0 + 32, 1 + 32 * rh:33 + 32 * rh, 2:66]
                    nc.vector.tensor_mul(dst, S[64:96, sl], ths[rh][64:96, :])

            def shift_copies(dsts, src, eng):
                """src: (buf, p0) base-group h [32, PH, PW]; dsts: list of
                (buf, p0, ky, kx).  A (ky,kx) shift is a constant offset
                s = ky*PW + kx in the flattened [PH*PW] spatial layout, so
                every copy is one contiguous run per partition (kx != 0
                wraps row ends into the never-read pad columns).  eng picks
                the HWDGE queue to avoid head-of-line blocking."""
                sbuf, sp = src
                NP = PH * PW
                sf = sbuf[sp:sp + 32].rearrange("p h w -> p (h w)")
                for buf, p0, ky, kx in dsts:
                    s = ky * PW + kx
                    df = buf[p0:p0 + 32].rearrange("p h w -> p (h w)")
                    if s >= 0:
                        eng.dma_start(out=df[:, 0:NP - s], in_=sf[:, s:NP])
                    else:
                        eng.dma_start(out=df[:, -s:NP], in_=sf[:, 0:NP + s])

            L0_PASSES = [(A, 105, w0[:, kxi * G4:(kxi + 1) * G4], kxi - 1)
                         for kxi in range(3)]
            LD_PASSES = [(A, 105, wd[:, kxi * G4:(kxi + 1) * G4], kxi - 1)
                         for kxi in range(3)]

            def l1_passes(p):
                return ([(Ba[p], 128, w1a[:, kxi * G4:(kxi + 1) * G4],
                          kxi - 1) for kxi in range(3)]
                        + [(Bb2[p], 128, w1b2[:], 0), (Bb[p], 64, w1bp[:], 0)])

            # ================= recurrent steps ===========================
            # Layer 1 runs one step behind layer 0, and the emission order
            # is software-pipelined: both layers' matmul+sigmoid bursts go
            # out first (keeping PE/ACT fed), then L0's vector chain, then
            # its h-muls (so the DVE never head-of-line blocks on a tanh),
            # then L1's.  L1(t-1)'s inputs (h0(t-1), h1(t-2)) are all ready
            # before L0(t) starts.  The h0(t) -> Ba/Bb/Bb2 copies are
            # emitted AFTER L1(t-1)'s matmuls so Tile keeps them WAR.
            def xload(t):
                # x_t into A's 3 ky-groups (ky=0 @32, ky=-1 @67, ky=+1 @102)
                nc.sync.dma_start(out=A[32:35, :, :], in_=xp_d[t])
                nc.sync.dma_start(out=A[67:70, 1:PH, :],
                                  in_=xp_d[t, :, 0:PH - 1, :])
                nc.sync.dma_start(out=A[102:105, 0:PH - 1, :],
                                  in_=xp_d[t, :, 1:PH, :])

            def l1_pointwise_copies(S1, p):
                # h1(t-1) goes into set p = t%2, read by L1(t) next step
                th1 = pointwise_core(S1, cst1)
                h_muls(S1, th1, (Ba[p], 96))
                shift_copies([(Ba[p], 32, -1, 0), (Bb[p], 32, 1, 0)],
                             (Ba[p], 96), nc.scalar)
                shift_copies([(Bb2[p], 32, 1, -1), (Bb2[p], 96, 1, 1)],
                             (Ba[p], 96), nc.sync)

            xload(0)
            S1 = None
            for t in range(T):
                p = t % 2
                S0 = sgate.tile([G4, NPIX], F16, tag="S", name="S0")
                conv_mm_sigma(L0_PASSES, b0, S0)
                if t > 0:
                    S1 = sgate.tile([G4, NPIX], F16, tag="S", name="S1")
                    conv_mm_sigma(l1_passes(1 - p), b1, S1)   # L1(t-1) gates
                if t + 1 < T:
                    xload(t + 1)       # prefetch; WAR-ordered after L0(t) mms

                # L0(t) pointwise + h0 into A, then h0 -> L1 rhs set t%2
                th0 = pointwise_core(S0, cst0)
                h_muls(S0, th0, (A, 0))
                shift_copies([(A, 35, -1, 0), (A, 70, 1, 0)], (A, 0), nc.sync)
                shift_copies([(Ba[p], 64, 0, 0), (Ba[p], 0, -1, 0),
                              (Bb[p], 0, 1, 0)], (A, 0), nc.scalar)
                shift_copies([(Bb2[p], 0, 1, -1), (Bb2[p], 64, 1, 1)],
                             (A, 0), nc.sync)

                if t > 0:
                    l1_pointwise_copies(S1, p)   # h1(t-1) -> set t%2

            p = T % 2
            S1 = sgate.tile([G4, NPIX], F16, tag="S", name="S1")
            conv_mm_sigma(l1_passes(1 - p), b1, S1)   # L1(T-1)
            l1_pointwise_copies(S1, p)

            # ================= decoder step ==============================
            shift_copies([(A, 0, 0, 0), (A, 35, -1, 0), (A, 70, 1, 0)],
                         (Ba[p], 96), nc.sync)
            Sd = sgate.tile([G4, NPIX], F16, tag="S")
            conv_mm_sigma(LD_PASSES, bd, Sd)
            thd = pointwise_core(Sd, cst1)
            h_muls(Sd, thd, hdc)

            # ================= FC head ===================================
            nc.sync.dma_start(out=a2a_in[:], in_=hdc[:])
            nc.gpsimd.collective_compute(
                "AllToAll", OP.bypass,
                replica_groups=[list(range(N_CORES))],
                ins=[a2a_in[:].opt()], outs=[a2a_out[:].opt()])
            # transposed load with K-index q = p*128 + k2:
            # ft[p, m, k2] = a2a_out[m, p, k2] -- contiguous 128-elem runs
            nc.sync.dma_start(
                out=ft[:],
                in_=a2a_out[:].rearrange("m p k -> p m k"))

            # fc1 weight streamed in 4 chunks through a 2-buffer pool
            # (too big to keep resident next to the double-buffered state)
            def fw_load(c):
                fwt = fwp.tile([128, 32 * 256], F16, tag="fw")
                for q in range(2):
                    sl = slice(q * 4096, (q + 1) * 4096)
                    nc.sync.dma_start(out=fwt[:, sl],
                                      in_=fw_d[:, c * 8192 + q * 4096:
                                               c * 8192 + (q + 1) * 4096])
                return fwt

            psz = psum.tile([8, 256], F32, tag="z")
            fwts = [fw_load(0), fw_load(1)]
            for c in range(4):
                fwt = fwts[c]
                for j in range(32):
                    k2 = 32 * c + j
                    nc.tensor.matmul(psz[:], ft[:, :, k2],
                                     fwt[:, j * 256:(j + 1) * 256],
                                     start=(k2 == 0), stop=(k2 == 127))
                if c + 2 < 4:
                    fwts.append(fw_load(c + 2))
            z1s = scr.tile([8, 256], F32, tag="z1")
            nc.vector.tensor_copy(z1s[:], psz[:])
            nc.sync.dma_start(out=z1part[:], in_=z1s[:])
            nc.gpsimd.collective_compute(
                "ReduceScatter", OP.add,
                replica_groups=[list(range(N_CORES))],
                ins=[z1part[:].opt()], outs=[z1red[:].opt()])

            zr = scr.tile([128, 2], F32, tag="zr")
            nc.sync.dma_start(out=zr[:],
                              in_=z1red[:].rearrange("(j p) -> p j", p=128))
            zrb = scr.tile([128, 2], F32, tag="zrb")
            nc.vector.tensor_add(zrb[:], zr[:], fb[:])
            h256 = scr.tile([128, 2], F16, tag="h256")
            nc.vector.tensor_scalar_max(h256[:], zrb[:], 0.0)

            ps2 = psum.tile([97, 1], F32, tag="z")
            for j in range(2):
                nc.tensor.matmul(ps2[:], w2[:, j * 97:(j + 1) * 97],
                                 h256[:, j:j + 1],
                                 start=(j == 0), stop=(j == 1))
            outs = scr.tile([97, 1], F32, tag="outs")
            nc.vector.tensor_add(outs[:], ps2[:], b2[:])
            nc.sync.dma_start(out=out_d[:], in_=outs[:])

    nc.compile()
    return nc


def _prep_inputs(x, Wenc0, benc0, Wenc1, benc1, Wdec, bdec,
                 fc1_w, fc1_b, fc2_w, fc2_b):
    """Host-side: pad/reorder/cast everything into device layouts."""
    f16 = np.float16

    def conv_w(Wk, reorder_x):
        # Wk [128, Cin, 3, 3] -> per-kx [ngrp*ch, 128] with ky stacked on
        # partitions; gate-g output channels pre-scaled x2.
        Wk = np.asarray(Wk, np.float32).copy()
        Wk[96:128] *= 2.0
        if reorder_x:  # [x(3), h(32)] -> [h(32), x(3)]
            Wk = np.concatenate([Wk[:, 3:], Wk[:, :3]], axis=1)
        cin = Wk.shape[1]
        out = np.zeros((3 * cin, 3 * G4), np.float32)
        for g, dy in enumerate((1, 0, 2)):   # group order ky = 0, -1, +1
            for kxi in range(3):
                # [cin, 128]
                out[g * cin:(g + 1) * cin, kxi * G4:(kxi + 1) * G4] = \
                    Wk[:, :, dy, kxi].T
        return out.astype(f16)

    def bias_v(b):
        b = np.asarray(b, np.float32).copy()
        b[96:128] *= 2.0
        return b.reshape(G4, 1)

    w0_full = conv_w(Wenc0, True)       # [105, 384]
    wd_full = conv_w(Wdec, True)
    w1_full = conv_w(Wenc1, False)      # [192, 384]; groups ky = 0, -1, +1
    # Ba's partition groups are ky=-1 @0-63, ky=0 @64-127
    w1a = np.ascontiguousarray(
        np.concatenate([w1_full[64:128], w1_full[0:64]], axis=0))
    # Bb2: (ky=+1,kx=-1) on p0-63, (ky=+1,kx=+1) on p64-127
    w1b2 = np.ascontiguousarray(
        np.concatenate([w1_full[128:192, 0:G4],
                        w1_full[128:192, 2 * G4:3 * G4]], axis=0))
    # Bb: (ky=+1, kx=0)
    w1bp = np.ascontiguousarray(w1_full[128:192, G4:2 * G4])

    xpad = np.zeros((B, T, C, PH, PW), f16)
    xpad[:, :, :, 1:65, 2:66] = np.asarray(x, np.float32)

    fc1_w = np.asarray(fc1_w, np.float32)
    fb = np.asarray(fc1_b, np.float32).reshape(2, 128).T.copy()  # [128, 2]
    w2 = np.asarray(fc2_w, np.float32).T.reshape(2, 128, 97)
    w2 = np.ascontiguousarray(w2.transpose(1, 0, 2)).reshape(128, 2 * 97)
    b2 = np.asarray(fc2_b, np.float32).reshape(97, 1)

    in_maps = []
    for k in range(N_CORES):
        w1k = fc1_w[:, k * KSL:(k + 1) * KSL].T            # [16384, 256]
        # K-index q = p*128 + k2  ->  fw[p, k2, n] = w1k[p*128 + k2, n]
        fwk = w1k.reshape(128, 128 * 256)
        in_maps.append({
            "xp": np.ascontiguousarray(xpad[k]),
            "w0": w0_full, "w1a": w1a.astype(f16),
            "w1b2": w1b2.astype(f16), "w1bp": w1bp.astype(f16),
            "wd": wd_full,
            "b0": bias_v(benc0), "b1": bias_v(benc1), "bd": bias_v(bdec),
            "fw": fwk.astype(f16), "fb": fb,
            "w2": w2.astype(f16), "b2": b2,
        })
    return in_maps


def kernel(**inputs):
    if "nc" not in _CACHE:
        _CACHE["nc"] = _build_nc()
    nc = _CACHE["nc"]
    in_maps = _prep_inputs(**inputs)
    res = run_bass_kernel_spmd(nc, in_maps, core_ids=list(range(N_CORES)),
                               trace=TRACE)
    _CACHE["last_result"] = res
    out = np.stack([res.results[k]["out"][:, 0] for k in range(N_CORES)])
    return out.astype(np.float32)
